# revision 16
# baseline (speedup 1.0000x reference)
"""Trainium2 Bass kernel for the 8-qubit variational-circuit batch evaluator.

Math (collapsed Kronecker product):
  With u_q = x_q^2, Z_q = 1+u_q, zz_q = 1+x_q^4, P27 = prod_{q=2..7} Z_q,
    A  = Z1 * P27,  BB = Z0*zz0*Z1*zz1,  w = x0*x1
    out = C0 + C1/sqrt(A) + C2*w/sqrt(BB) + C3*w*u1/sqrt(BB*P27)

Data-parallel: batch 131072 x 8 sharded across 8 cores (16384 rows each),
[128 partitions x 128 rows x 8 feats] per core.

Performance structure:
  * C0..C3 derive only from the 3 rotation weights (O(1) host work) and are
    baked into the program as immediates at build time (the NEFF is compiled
    inside kernel(), when the weights are known). No coeff DMA, no slow
    pointer-scalar TensorScalar ops.
  * Scratch is SLOT-MAJOR [p, slot, row]: ops stream contiguous row-runs
    per slot instead of paying per-row AP-iteration overhead; bf16
    intermediates give 2x/4x DVE modes (tolerance 2e-2, measured ~2e-3).
  * SACRIFICIAL ROW PADDING (PAD=16): every compute op processes 16 rows
    past its chunk. Hardware-validated finding: the last <=32 bytes of a
    contiguous per-partition write stream can be read stale by a
    downstream consumer even ~1us later (write-combine tail retention;
    semaphore slack does NOT fix it). Padding keeps real data out of every
    stream tail. Padded rows read garbage and produce garbage, but only
    rows 0:128 are ever DMA'd out.
  * A warm-up execution inside kernel() further pins first-exec state.
  * GPSIMD unused: bf16 tensor ops there run a ~14 ns/elem software loop
    and saturate the SBUF port shared with the DVE.
"""

import numpy as np

import concourse.bass as bass
from concourse import mybir
from concourse.bass_utils import run_bass_kernel_spmd

N_CORES = 8
BATCH = 131072
NQ = 8
B_LOCAL = BATCH // N_CORES  # 16384
P = 128
R = B_LOCAL // P            # 128 rows per partition
PAD = 16                    # sacrificial rows per op (see module docstring)
RP = R + PAD
CHUNKS = [(0, 32), (32, 96)]  # (row offset, rows): small first chunk starts compute early

F32 = mybir.dt.float32
BF16 = mybir.dt.bfloat16
AF = mybir.ActivationFunctionType
ALU = mybir.AluOpType

# s slot map (bf16, slot-major [p, slot, RP]):
#  0:8  u_q = x_q^2            20:25  pp = [Z0Z1, Z2Z3, Z4Z5, Z6Z7, zzp]
#  8:10 v = x0^4, x1^4         25:27  [Z2345, BB]
# 10:20 z = [Z0..Z7,zz0,zz1]   27 P27 ; 28 A ; 32 w
# 33:35 [u1*K, w*R2] ; 35 f2 ; 37 f4
# s2 (fp32): 0:3 rsqrt out [R2, K, R1] ; 3 f5
NS = 38


def _act_raw(nc, se, out, in_, func):
    """InstActivation without bass's Rsqrt accuracy guard (validated on HW)."""
    b = nc.const_aps.scalar_like(0.0, in_)
    ins = [se.lower_ap(in_), se.lower_ap(b),
           mybir.ImmediateValue(dtype=mybir.dt.float32, value=1.0),
           mybir.ImmediateValue(dtype=mybir.dt.float32, value=0.0)]
    return se.add_instruction(mybir.InstActivation(
        name=nc.get_next_instruction_name(), func=func,
        ins=ins, outs=[se.lower_ap(out)]))


def _build_nc(co):
    c0f, c1f, c2f, c3f = (float(v) for v in co)
    nc = bass.Bass()
    x = nc.declare_dram_parameter("x", [B_LOCAL, NQ], F32, isOutput=False)
    y = nc.declare_dram_parameter("y", [B_LOCAL], F32, isOutput=True)

    xv = x.rearrange("(p r) q -> p r q", p=P)      # [128, 128, 8]
    yv = y.rearrange("(p r) -> p r", p=P)          # [128, 128]

    import contextlib
    with contextlib.ExitStack() as ctx:
        xt = ctx.enter_context(nc.sbuf_tensor("xt", [P, RP, NQ], F32))
        s = ctx.enter_context(nc.sbuf_tensor("s", [P, NS, RP], BF16))
        s2 = ctx.enter_context(nc.sbuf_tensor("s2", [P, 5, RP], F32))
        ot = ctx.enter_context(nc.sbuf_tensor("ot", [P, RP], F32))
        junk = ctx.enter_context(nc.sbuf_tensor("junk", [P, 2], F32))

        s_in0 = ctx.enter_context(nc.semaphore("s_in0"))
        s_in1 = ctx.enter_context(nc.semaphore("s_in1"))
        s_dve1 = ctx.enter_context(nc.semaphore("s_dve1"))
        s_rsq = ctx.enter_context(nc.semaphore("s_rsq"))
        s_out = ctx.enter_context(nc.semaphore("s_out"))
        block = ctx.enter_context(nc.Block())

        def rows(c):
            r0, rc = CHUNKS[c]
            return r0, r0 + rc

        def rowsp(c):
            r0, rc = CHUNKS[c]
            return r0, r0 + rc + PAD

        @block.sync
        def _(sync):
            for c in range(len(CHUNKS)):
                r0, r1 = rows(c)
                sync.dma_start(
                    out=xt[:, r0:r1, :], in_=xv[:, r0:r1, :],
                ).then_inc(s_in0 if c == 0 else s_in1, 16)
            sync.wait_ge(s_out, 1)
            sync.dma_start(out=yv[:, :], in_=ot[:, 0:R]).then_inc(s_in0, 16)

        @block.scalar
        def _(scalar):
            # table prefetch while the input DMA is in flight
            _act_raw(nc, scalar, junk[:, 1:2], junk[:, 0:1], AF.Rsqrt)
            for c in range(len(CHUNKS)):
                r0, r1 = rowsp(c)
                scalar.wait_ge(s_dve1, c + 1)
                # [BB, P27, A] -> [R2, K, R1] (fp32 out)
                _act_raw(nc, scalar, s2[:, 0:3, r0:r1],
                         s[:, 26:29, r0:r1], AF.Rsqrt)
                # f5 = C1*R1 + C0 on ACT (Copy takes float scale/bias);
                # same-engine read of R1, and fences the rsqrt's writes
                scalar.activation(s2[:, 3:4, r0:r1], s2[:, 2:3, r0:r1],
                                  AF.Copy, bias=c0f,
                                  scale=c1f).then_inc(s_rsq, 1)

        @block.vector
        def _(vector):
            for c in range(len(CHUNKS)):
                r0, r1 = rowsp(c)
                vector.wait_ge(s_in0 if c == 0 else s_in1, 16)
                xc = xt[:, r0:r1, :].rearrange("p r q -> p q r")
                # u = x^2: strided fp32 read (~2 cyc/elem), contiguous
                # slot-major bf16 write (a transposed write is 2x worse)
                vector.tensor_mul(s[:, 0:8, r0:r1], xc, xc)
                # v = u01^2
                vector.tensor_mul(s[:, 8:10, r0:r1],
                                  s[:, 0:2, r0:r1], s[:, 0:2, r0:r1])
                # z = [u | v] + 1
                vector.tensor_scalar(s[:, 10:20, r0:r1], s[:, 0:10, r0:r1],
                                     1.0, None, ALU.add)
                # pp = [Z0Z1, Z2Z3, Z4Z5, Z6Z7, zzp]
                vector.tensor_mul(s[:, 20:25, r0:r1],
                                  s[:, 10:20:2, r0:r1],
                                  s[:, 11:20:2, r0:r1])
                # [Z2345, BB] = [Z23, Z01] * [Z45, zzp]
                vector.tensor_mul(s[:, 25:27, r0:r1],
                                  s[:, 21:19:-1, r0:r1],
                                  s[:, 22:25:2, r0:r1])
                # P27 = Z2345 * Z67
                vector.tensor_mul(s[:, 27:28, r0:r1],
                                  s[:, 25:26, r0:r1],
                                  s[:, 23:24, r0:r1])
                # A = P27 * Z1
                vector.tensor_mul(s[:, 28:29, r0:r1],
                                  s[:, 27:28, r0:r1],
                                  s[:, 11:12, r0:r1])
                # w = x0 * x1; completion also spaces A's writeback from
                # ACT's read of slots 26:29
                vector.tensor_mul(
                    s[:, 32:33, r0:r1],
                    xt[:, r0:r1, 0:1].rearrange("p r q -> p q r"),
                    xt[:, r0:r1, 1:2].rearrange("p r q -> p q r"),
                ).then_inc(s_dve1, 1)
            # merged combine over all (padded) rows
            vector.wait_ge(s_rsq, len(CHUNKS))
            # [u1*K, w*R2]: in0 = s slots {1, 32}, in1 = s2 slots {1, 0}
            vector.tensor_mul(s[:, 33:35, :],
                              s[:, 1:33:31, :], s2[:, 1::-1, :])
            # f2 = C3*(u1 K) + C2  (immediates)
            vector.tensor_scalar(s[:, 35:36, :], s[:, 33:34, :],
                                 c3f, c2f, ALU.mult, ALU.add)
            # f4 = (w R2) * f2
            vector.tensor_mul(s[:, 37:38, :], s[:, 34:35, :], s[:, 35:36, :])
            # out = f4 + f5 (fp32 out)
            vector.tensor_add(
                ot[:, :],
                s[:, 37:38, :].rearrange("p one r -> p (one r)"),
                s2[:, 3:4, :].rearrange("p one r -> p (one r)"),
            ).then_inc(s_out, 1)

    return nc


_NC = None
_NC_KEY = None


def _get_nc(co):
    global _NC, _NC_KEY
    key = tuple(np.asarray(co, np.float64).tolist())
    if _NC is None or _NC_KEY != key:
        _NC = _build_nc(co)
        _NC_KEY = key
    return _NC


def _host_coeffs(weights_re, weights_im):
    w = (np.asarray(weights_re, np.float64)
         + 1j * np.asarray(weights_im, np.float64)) * 0.5
    c, s = np.cos(w), np.sin(w)

    def rymat(i):
        return np.array([[c[i], -s[i]], [s[i], c[i]]])

    rot = rymat(2) @ (rymat(1) @ rymat(0))
    A, B = rot[0, 0], rot[0, 1]
    alpha = abs(B) ** 2
    beta = abs(A) ** 2 - abs(B) ** 2
    gam = A * np.conj(B)
    return np.array([alpha + beta / 2, beta / 2, gam.real, gam.imag],
                    dtype=np.float32)


def kernel(inputs, weights_re, weights_im):
    x = np.ascontiguousarray(np.asarray(inputs, dtype=np.float32))
    co = _host_coeffs(weights_re, weights_im)
    nc = _get_nc(co)
    shards = np.split(x, N_CORES, axis=0)
    in_maps = [{"x": sh} for sh in shards]
    # warm-up execution: pins device state (ACT tables, DMA paths) so the
    # returned result always comes from a steady-state execution
    run_bass_kernel_spmd(nc, in_maps, list(range(N_CORES)))
    res = run_bass_kernel_spmd(nc, in_maps, list(range(N_CORES)))
    return np.concatenate([res.results[i]["y"] for i in range(N_CORES)])


# revision 17
# speedup vs baseline: 1.0432x; 1.0432x over previous
"""Trainium2 Bass kernel for the 8-qubit variational-circuit batch evaluator.

Math (collapsed Kronecker product):
  With u_q = x_q^2, Z_q = 1+u_q, zz_q = 1+x_q^4, P27 = prod_{q=2..7} Z_q,
    A  = Z1 * P27,  BB = Z0*zz0*Z1*zz1,  w = x0*x1
    out = C0 + C1/sqrt(A) + C2*w/sqrt(BB) + C3*w*u1/sqrt(BB*P27)

Data-parallel: batch 131072 x 8 sharded across 8 cores (16384 rows each),
[128 partitions x 128 rows x 8 feats] per core.

Performance structure:
  * C0..C3 derive only from the 3 rotation weights (O(1) host work) and are
    baked into the program as immediates at build time (the NEFF is compiled
    inside kernel(), when the weights are known). No coeff DMA, no slow
    pointer-scalar TensorScalar ops.
  * Scratch is SLOT-MAJOR [p, slot, row]: ops stream contiguous row-runs
    per slot instead of paying per-row AP-iteration overhead; bf16
    intermediates give 2x/4x DVE modes (tolerance 2e-2, measured ~2e-3).
  * SACRIFICIAL ROW PADDING (PAD=16): every compute op processes 16 rows
    past its chunk. Hardware-validated finding: the last <=32 bytes of a
    contiguous per-partition write stream can be read stale by a
    downstream consumer even ~1us later (write-combine tail retention;
    semaphore slack does NOT fix it). Padding keeps real data out of every
    stream tail. Padded rows read garbage and produce garbage, but only
    rows 0:128 are ever DMA'd out.
  * A warm-up execution inside kernel() further pins first-exec state.
  * GPSIMD unused: bf16 tensor ops there run a ~14 ns/elem software loop
    and saturate the SBUF port shared with the DVE.
"""

import numpy as np

import concourse.bass as bass
from concourse import mybir
from concourse.bass_utils import run_bass_kernel_spmd

N_CORES = 8
BATCH = 131072
NQ = 8
B_LOCAL = BATCH // N_CORES  # 16384
P = 128
R = B_LOCAL // P            # 128 rows per partition
PAD = 16                    # sacrificial rows per op (see module docstring)
RP = R + PAD
CHUNKS = [(0, 32), (32, 96)]  # (row offset, rows): small first chunk starts compute early

F32 = mybir.dt.float32
BF16 = mybir.dt.bfloat16
AF = mybir.ActivationFunctionType
ALU = mybir.AluOpType

# s slot map (bf16, slot-major [p, slot, RP]):
#  0:4  u0..u3 = x_q^2 (DVE)   20:25  pp = [Z01, Z23, zzp, Z45, Z67]
#  4:6  v = x0^4, x1^4         25:27  [Z2345, BB]
# 10:20 z = [Z0,Z1,Z2,Z3,zz0,zz1,Z4,Z5,Z6,Z7]
# 27 P27 ; 28 A ; 32 w ; 33:35 [u1*K, w*R2] ; 35 f2 ; 37 f4
# u4..u7 live in per-chunk fp32 tiles xt4a/xt4b (ACT-written; private
# tiles keep ACT's write streams disjoint from anything another agent
# reads near its tail)
# s2 (fp32): 0:3 rsqrt out [R2, K, R1] ; 3 f5
NS = 38


def _act_raw(nc, se, out, in_, func):
    """InstActivation without bass's Rsqrt accuracy guard (validated on HW)."""
    b = nc.const_aps.scalar_like(0.0, in_)
    ins = [se.lower_ap(in_), se.lower_ap(b),
           mybir.ImmediateValue(dtype=mybir.dt.float32, value=1.0),
           mybir.ImmediateValue(dtype=mybir.dt.float32, value=0.0)]
    return se.add_instruction(mybir.InstActivation(
        name=nc.get_next_instruction_name(), func=func,
        ins=ins, outs=[se.lower_ap(out)]))


def _build_nc(co):
    c0f, c1f, c2f, c3f = (float(v) for v in co)
    nc = bass.Bass()
    x = nc.declare_dram_parameter("x", [B_LOCAL, NQ], F32, isOutput=False)
    y = nc.declare_dram_parameter("y", [B_LOCAL], F32, isOutput=True)

    xv = x.rearrange("(p r) q -> p r q", p=P)      # [128, 128, 8]
    yv = y.rearrange("(p r) -> p r", p=P)          # [128, 128]

    import contextlib
    with contextlib.ExitStack() as ctx:
        xt = ctx.enter_context(nc.sbuf_tensor("xt", [P, RP, NQ], F32))
        s = ctx.enter_context(nc.sbuf_tensor("s", [P, NS, RP], BF16))
        s2 = ctx.enter_context(nc.sbuf_tensor("s2", [P, 5, RP], F32))
        ot = ctx.enter_context(nc.sbuf_tensor("ot", [P, RP], F32))
        junk = ctx.enter_context(nc.sbuf_tensor("junk", [P, 2], F32))
        xt4 = [ctx.enter_context(nc.sbuf_tensor(
            f"xt4_{c}", [P, 4, CHUNKS[c][1] + PAD], F32))
            for c in range(len(CHUNKS))]

        s_in0 = ctx.enter_context(nc.semaphore("s_in0"))
        s_in1 = ctx.enter_context(nc.semaphore("s_in1"))
        s_dve1 = ctx.enter_context(nc.semaphore("s_dve1"))
        s_asq = ctx.enter_context(nc.semaphore("s_asq"))
        s_rsq = ctx.enter_context(nc.semaphore("s_rsq"))
        s_out = ctx.enter_context(nc.semaphore("s_out"))
        block = ctx.enter_context(nc.Block())

        def rows(c):
            r0, rc = CHUNKS[c]
            return r0, r0 + rc

        def rowsp(c):
            r0, rc = CHUNKS[c]
            return r0, r0 + rc + PAD

        @block.sync
        def _(sync):
            for c in range(len(CHUNKS)):
                r0, r1 = rows(c)
                sync.dma_start(
                    out=xt[:, r0:r1, :], in_=xv[:, r0:r1, :],
                ).then_inc(s_in0 if c == 0 else s_in1, 16)
            sync.wait_ge(s_out, 1)
            sync.dma_start(out=yv[:, :], in_=ot[:, 0:R]).then_inc(s_in0, 16)

        @block.scalar
        def _(scalar):
            # table prefetch while the input DMA is in flight
            _act_raw(nc, scalar, junk[:, 1:2], junk[:, 0:1], AF.Rsqrt)
            # squares for q=4..7 (the DVE does q=0..3 concurrently);
            # fp32 out into chunk-private tiles
            for c in range(len(CHUNKS)):
                r0, r1 = rowsp(c)
                scalar.wait_ge(s_in0 if c == 0 else s_in1, 16)
                scalar.activation(
                    xt4[c][:, :, :],
                    xt[:, r0:r1, 4:8].rearrange("p r q -> p q r"),
                    AF.Square).then_inc(s_asq, 1)
            for c in range(len(CHUNKS)):
                r0, r1 = rowsp(c)
                scalar.wait_ge(s_dve1, c + 1)
                # [BB, P27, A] -> [R2, K, R1] (fp32 out)
                _act_raw(nc, scalar, s2[:, 0:3, r0:r1],
                         s[:, 26:29, r0:r1], AF.Rsqrt)
                # f5 = C1*R1 + C0 on ACT (Copy takes float scale/bias);
                # same-engine read of R1, and fences the rsqrt's writes
                scalar.activation(s2[:, 3:4, r0:r1], s2[:, 2:3, r0:r1],
                                  AF.Copy, bias=c0f,
                                  scale=c1f).then_inc(s_rsq, 1)

        @block.vector
        def _(vector):
            for c in range(len(CHUNKS)):
                r0, r1 = rowsp(c)
                vector.wait_ge(s_in0 if c == 0 else s_in1, 16)
                xc = xt[:, r0:r1, 0:4].rearrange("p r q -> p q r")
                # u0..u3 = x^2 (ACT squares q=4..7 concurrently)
                vector.tensor_mul(s[:, 0:4, r0:r1], xc, xc)
                # v = u01^2
                vector.tensor_mul(s[:, 4:6, r0:r1],
                                  s[:, 0:2, r0:r1], s[:, 0:2, r0:r1])
                # z[0:6] = [Z0, Z1, Z2, Z3, zz0, zz1] = [u0..u3, v] + 1
                vector.tensor_scalar(s[:, 10:16, r0:r1], s[:, 0:6, r0:r1],
                                     1.0, None, ALU.add)
                # z[6:10] = [Z4..Z7] = u4..u7 + 1 (from ACT's fp32 tile)
                vector.wait_ge(s_asq, c + 1)
                vector.tensor_scalar(s[:, 16:20, r0:r1], xt4[c][:, :, :],
                                     1.0, None, ALU.add)
                # pp = [Z01, Z23, zzp, Z45, Z67]
                vector.tensor_mul(s[:, 20:25, r0:r1],
                                  s[:, 10:20:2, r0:r1],
                                  s[:, 11:20:2, r0:r1])
                # [Z2345, BB] = [Z23, Z01] * [Z45, zzp]
                vector.tensor_mul(s[:, 25:27, r0:r1],
                                  s[:, 21:19:-1, r0:r1],
                                  s[:, 23:21:-1, r0:r1])
                # P27 = Z2345 * Z67
                vector.tensor_mul(s[:, 27:28, r0:r1],
                                  s[:, 25:26, r0:r1],
                                  s[:, 24:25, r0:r1])
                # A = P27 * Z1
                vector.tensor_mul(s[:, 28:29, r0:r1],
                                  s[:, 27:28, r0:r1],
                                  s[:, 11:12, r0:r1])
                # w = x0 * x1; completion also spaces A's writeback from
                # ACT's read of slots 26:29
                vector.tensor_mul(
                    s[:, 32:33, r0:r1],
                    xt[:, r0:r1, 0:1].rearrange("p r q -> p q r"),
                    xt[:, r0:r1, 1:2].rearrange("p r q -> p q r"),
                ).then_inc(s_dve1, 1)
            # merged combine over all (padded) rows
            vector.wait_ge(s_rsq, len(CHUNKS))
            # [u1*K, w*R2]: in0 = s slots {1, 32}, in1 = s2 slots {1, 0}
            vector.tensor_mul(s[:, 33:35, :],
                              s[:, 1:33:31, :], s2[:, 1::-1, :])
            # f2 = C3*(u1 K) + C2  (immediates)
            vector.tensor_scalar(s[:, 35:36, :], s[:, 33:34, :],
                                 c3f, c2f, ALU.mult, ALU.add)
            # f4 = (w R2) * f2
            vector.tensor_mul(s[:, 37:38, :], s[:, 34:35, :], s[:, 35:36, :])
            # out = f4 + f5 (fp32 out)
            vector.tensor_add(
                ot[:, :],
                s[:, 37:38, :].rearrange("p one r -> p (one r)"),
                s2[:, 3:4, :].rearrange("p one r -> p (one r)"),
            ).then_inc(s_out, 1)

    return nc


_NC = None
_NC_KEY = None


def _get_nc(co):
    global _NC, _NC_KEY
    key = tuple(np.asarray(co, np.float64).tolist())
    if _NC is None or _NC_KEY != key:
        _NC = _build_nc(co)
        _NC_KEY = key
    return _NC


def _host_coeffs(weights_re, weights_im):
    w = (np.asarray(weights_re, np.float64)
         + 1j * np.asarray(weights_im, np.float64)) * 0.5
    c, s = np.cos(w), np.sin(w)

    def rymat(i):
        return np.array([[c[i], -s[i]], [s[i], c[i]]])

    rot = rymat(2) @ (rymat(1) @ rymat(0))
    A, B = rot[0, 0], rot[0, 1]
    alpha = abs(B) ** 2
    beta = abs(A) ** 2 - abs(B) ** 2
    gam = A * np.conj(B)
    return np.array([alpha + beta / 2, beta / 2, gam.real, gam.imag],
                    dtype=np.float32)


def kernel(inputs, weights_re, weights_im):
    x = np.ascontiguousarray(np.asarray(inputs, dtype=np.float32))
    co = _host_coeffs(weights_re, weights_im)
    nc = _get_nc(co)
    shards = np.split(x, N_CORES, axis=0)
    in_maps = [{"x": sh} for sh in shards]
    # warm-up execution: pins device state (ACT tables, DMA paths) so the
    # returned result always comes from a steady-state execution
    run_bass_kernel_spmd(nc, in_maps, list(range(N_CORES)))
    res = run_bass_kernel_spmd(nc, in_maps, list(range(N_CORES)))
    return np.concatenate([res.results[i]["y"] for i in range(N_CORES)])


# revision 18
# speedup vs baseline: 1.1084x; 1.0625x over previous
"""Trainium2 Bass kernel for the 8-qubit variational-circuit batch evaluator.

Math (collapsed Kronecker product):
  With u_q = x_q^2, Z_q = 1+u_q, zz_q = 1+x_q^4, P27 = prod_{q=2..7} Z_q,
    A  = Z1 * P27,  BB = Z0*zz0*Z1*zz1,  w = x0*x1
    out = C0 + C1/sqrt(A) + C2*w/sqrt(BB) + C3*w*u1/sqrt(BB*P27)

Data-parallel: batch 131072 x 8 sharded across 8 cores (16384 rows each),
[128 partitions x 128 rows x 8 feats] per core.

Performance structure:
  * C0..C3 derive only from the 3 rotation weights (O(1) host work) and are
    baked into the program as immediates at build time (the NEFF is compiled
    inside kernel(), when the weights are known). No coeff DMA, no slow
    pointer-scalar TensorScalar ops.
  * Scratch is SLOT-MAJOR [p, slot, row]: ops stream contiguous row-runs
    per slot instead of paying per-row AP-iteration overhead; bf16
    intermediates give 2x/4x DVE modes (tolerance 2e-2, measured ~2e-3).
  * SACRIFICIAL ROW PADDING (PAD=16): every compute op processes 16 rows
    past its chunk. Hardware-validated finding: the last <=32 bytes of a
    contiguous per-partition write stream can be read stale by a
    downstream consumer even ~1us later (write-combine tail retention;
    semaphore slack does NOT fix it). Padding keeps real data out of every
    stream tail. Padded rows read garbage and produce garbage, but only
    rows 0:128 are ever DMA'd out.
  * A warm-up execution inside kernel() further pins first-exec state.
  * GPSIMD unused: bf16 tensor ops there run a ~14 ns/elem software loop
    and saturate the SBUF port shared with the DVE.
"""

import numpy as np

import concourse.bass as bass
from concourse import mybir
from concourse.bass_utils import run_bass_kernel_spmd

N_CORES = 8
BATCH = 131072
NQ = 8
B_LOCAL = BATCH // N_CORES  # 16384
P = 128
R = B_LOCAL // P            # 128 rows per partition
PAD = 16                    # sacrificial rows per op (see module docstring)
RP = R + PAD
CHUNKS = [(0, 32), (32, 96)]  # (row offset, rows): small first chunk starts compute early

F32 = mybir.dt.float32
BF16 = mybir.dt.bfloat16
AF = mybir.ActivationFunctionType
ALU = mybir.AluOpType

# s slot map (bf16, slot-major [p, slot, RP]):
#  0:4  u0..u3 = x_q^2 (DVE)   20:25  pp = [Z01, Z23, zzp, Z45, Z67]
#  4:6  v = x0^4, x1^4         25:27  [Z2345, BB]
# 10:20 z = [Z0,Z1,Z2,Z3,zz0,zz1,Z4,Z5,Z6,Z7]
# 27 P27 ; 28 A ; 32 w ; 33:35 [u1*K, w*R2] ; 35 f2 ; 37 f4
# u4..u7 live in per-chunk fp32 tiles xt4a/xt4b (ACT-written; private
# tiles keep ACT's write streams disjoint from anything another agent
# reads near its tail)
# s2 (fp32): 0:3 rsqrt out [R2, K, R1] ; 3 f5
NS = 38


def _act_raw(nc, se, out, in_, func):
    """InstActivation without bass's Rsqrt accuracy guard (validated on HW)."""
    b = nc.const_aps.scalar_like(0.0, in_)
    ins = [se.lower_ap(in_), se.lower_ap(b),
           mybir.ImmediateValue(dtype=mybir.dt.float32, value=1.0),
           mybir.ImmediateValue(dtype=mybir.dt.float32, value=0.0)]
    return se.add_instruction(mybir.InstActivation(
        name=nc.get_next_instruction_name(), func=func,
        ins=ins, outs=[se.lower_ap(out)]))


def _build_nc(co):
    c0f, c1f, c2f, c3f = (float(v) for v in co)
    nc = bass.Bass()
    x = nc.declare_dram_parameter("x", [B_LOCAL, NQ], F32, isOutput=False)
    y = nc.declare_dram_parameter("y", [B_LOCAL], F32, isOutput=True)

    xv = x.rearrange("(p r) q -> p r q", p=P)      # [128, 128, 8]
    yv = y.rearrange("(p r) -> p r", p=P)          # [128, 128]

    import contextlib
    with contextlib.ExitStack() as ctx:
        xt = ctx.enter_context(nc.sbuf_tensor("xt", [P, RP, NQ], F32))
        s = ctx.enter_context(nc.sbuf_tensor("s", [P, NS, RP], BF16))
        s2 = ctx.enter_context(nc.sbuf_tensor("s2", [P, 5, RP], F32))
        ot = ctx.enter_context(nc.sbuf_tensor("ot", [P, RP], F32))
        junk = ctx.enter_context(nc.sbuf_tensor("junk", [P, 2], F32))
        xt4 = [ctx.enter_context(nc.sbuf_tensor(
            f"xt4_{c}", [P, 4, CHUNKS[c][1] + PAD], F32))
            for c in range(len(CHUNKS))]

        s_in0 = ctx.enter_context(nc.semaphore("s_in0"))
        s_in1 = ctx.enter_context(nc.semaphore("s_in1"))
        s_dve1 = ctx.enter_context(nc.semaphore("s_dve1"))   # P27 ready
        s_dveA = ctx.enter_context(nc.semaphore("s_dveA"))   # A ready (via w)
        s_asq = ctx.enter_context(nc.semaphore("s_asq"))
        s_rsqa = ctx.enter_context(nc.semaphore("s_rsqa"))   # [R2,K] ready
        s_rsq = ctx.enter_context(nc.semaphore("s_rsq"))
        s_out = ctx.enter_context(nc.semaphore("s_out"))
        block = ctx.enter_context(nc.Block())

        def rows(c):
            r0, rc = CHUNKS[c]
            return r0, r0 + rc

        def rowsp(c):
            r0, rc = CHUNKS[c]
            return r0, r0 + rc + PAD

        @block.sync
        def _(sync):
            for c in range(len(CHUNKS)):
                r0, r1 = rows(c)
                sync.dma_start(
                    out=xt[:, r0:r1, :], in_=xv[:, r0:r1, :],
                ).then_inc(s_in0 if c == 0 else s_in1, 16)
            sync.wait_ge(s_out, 1)
            sync.dma_start(out=yv[:, :], in_=ot[:, 0:R]).then_inc(s_in0, 16)

        @block.scalar
        def _(scalar):
            # table prefetch while the input DMA is in flight
            _act_raw(nc, scalar, junk[:, 1:2], junk[:, 0:1], AF.Rsqrt)
            # squares for q=4..7 (the DVE does q=0..3 concurrently);
            # fp32 out into chunk-private tiles
            for c in range(len(CHUNKS)):
                r0, r1 = rowsp(c)
                scalar.wait_ge(s_in0 if c == 0 else s_in1, 16)
                scalar.activation(
                    xt4[c][:, :, :],
                    xt[:, r0:r1, 4:8].rearrange("p r q -> p q r"),
                    AF.Square).then_inc(s_asq, 1)
            for c in range(len(CHUNKS)):
                r0, r1 = rowsp(c)
                # [BB, P27] -> [R2, K] as soon as P27 lands (the combine's
                # first op only needs these two)
                scalar.wait_ge(s_dve1, c + 1)
                _act_raw(nc, scalar, s2[:, 0:2, r0:r1],
                         s[:, 26:28, r0:r1], AF.Rsqrt).then_inc(s_rsqa, 1)
                # A -> R1, then f5 = C1*R1 + C0 (Copy with float scale/bias)
                scalar.wait_ge(s_dveA, c + 1)
                _act_raw(nc, scalar, s2[:, 2:3, r0:r1],
                         s[:, 28:29, r0:r1], AF.Rsqrt)
                scalar.activation(s2[:, 3:4, r0:r1], s2[:, 2:3, r0:r1],
                                  AF.Copy, bias=c0f,
                                  scale=c1f).then_inc(s_rsq, 1)

        @block.vector
        def _(vector):
            for c in range(len(CHUNKS)):
                r0, r1 = rowsp(c)
                vector.wait_ge(s_in0 if c == 0 else s_in1, 16)
                xc = xt[:, r0:r1, 0:4].rearrange("p r q -> p q r")
                # u0..u3 = x^2 (ACT squares q=4..7 concurrently)
                vector.tensor_mul(s[:, 0:4, r0:r1], xc, xc)
                # v = u01^2
                vector.tensor_mul(s[:, 4:6, r0:r1],
                                  s[:, 0:2, r0:r1], s[:, 0:2, r0:r1])
                # z[0:6] = [Z0, Z1, Z2, Z3, zz0, zz1] = [u0..u3, v] + 1
                vector.tensor_scalar(s[:, 10:16, r0:r1], s[:, 0:6, r0:r1],
                                     1.0, None, ALU.add)
                # z[6:10] = [Z4..Z7] = u4..u7 + 1 (from ACT's fp32 tile)
                vector.wait_ge(s_asq, c + 1)
                vector.tensor_scalar(s[:, 16:20, r0:r1], xt4[c][:, :, :],
                                     1.0, None, ALU.add)
                # pp = [Z01, Z23, zzp, Z45, Z67]
                vector.tensor_mul(s[:, 20:25, r0:r1],
                                  s[:, 10:20:2, r0:r1],
                                  s[:, 11:20:2, r0:r1])
                # [Z2345, BB] = [Z23, Z01] * [Z45, zzp]
                vector.tensor_mul(s[:, 25:27, r0:r1],
                                  s[:, 21:19:-1, r0:r1],
                                  s[:, 23:21:-1, r0:r1])
                # P27 = Z2345 * Z67
                vector.tensor_mul(s[:, 27:28, r0:r1],
                                  s[:, 25:26, r0:r1],
                                  s[:, 24:25, r0:r1]).then_inc(s_dve1, 1)
                # A = P27 * Z1
                vector.tensor_mul(s[:, 28:29, r0:r1],
                                  s[:, 27:28, r0:r1],
                                  s[:, 11:12, r0:r1])
                # w = x0 * x1; completion also spaces A's writeback from
                # ACT's read of slots 26:29
                vector.tensor_mul(
                    s[:, 32:33, r0:r1],
                    xt[:, r0:r1, 0:1].rearrange("p r q -> p q r"),
                    xt[:, r0:r1, 1:2].rearrange("p r q -> p q r"),
                ).then_inc(s_dveA, 1)
            # merged combine over all (padded) rows; the first three ops
            # need only [R2, K], the final add also needs f5
            vector.wait_ge(s_rsqa, len(CHUNKS))
            # [u1*K, w*R2]: in0 = s slots {1, 32}, in1 = s2 slots {1, 0}
            vector.tensor_mul(s[:, 33:35, :],
                              s[:, 1:33:31, :], s2[:, 1::-1, :])
            # f2 = C3*(u1 K) + C2  (immediates)
            vector.tensor_scalar(s[:, 35:36, :], s[:, 33:34, :],
                                 c3f, c2f, ALU.mult, ALU.add)
            # f4 = (w R2) * f2
            vector.tensor_mul(s[:, 37:38, :], s[:, 34:35, :], s[:, 35:36, :])
            # out = f4 + f5 (fp32 out)
            vector.wait_ge(s_rsq, len(CHUNKS))
            vector.tensor_add(
                ot[:, :],
                s[:, 37:38, :].rearrange("p one r -> p (one r)"),
                s2[:, 3:4, :].rearrange("p one r -> p (one r)"),
            ).then_inc(s_out, 1)

    return nc


_NC = None
_NC_KEY = None


def _get_nc(co):
    global _NC, _NC_KEY
    key = tuple(np.asarray(co, np.float64).tolist())
    if _NC is None or _NC_KEY != key:
        _NC = _build_nc(co)
        _NC_KEY = key
    return _NC


def _host_coeffs(weights_re, weights_im):
    w = (np.asarray(weights_re, np.float64)
         + 1j * np.asarray(weights_im, np.float64)) * 0.5
    c, s = np.cos(w), np.sin(w)

    def rymat(i):
        return np.array([[c[i], -s[i]], [s[i], c[i]]])

    rot = rymat(2) @ (rymat(1) @ rymat(0))
    A, B = rot[0, 0], rot[0, 1]
    alpha = abs(B) ** 2
    beta = abs(A) ** 2 - abs(B) ** 2
    gam = A * np.conj(B)
    return np.array([alpha + beta / 2, beta / 2, gam.real, gam.imag],
                    dtype=np.float32)


def kernel(inputs, weights_re, weights_im):
    x = np.ascontiguousarray(np.asarray(inputs, dtype=np.float32))
    co = _host_coeffs(weights_re, weights_im)
    nc = _get_nc(co)
    shards = np.split(x, N_CORES, axis=0)
    in_maps = [{"x": sh} for sh in shards]
    # warm-up execution: pins device state (ACT tables, DMA paths) so the
    # returned result always comes from a steady-state execution
    run_bass_kernel_spmd(nc, in_maps, list(range(N_CORES)))
    res = run_bass_kernel_spmd(nc, in_maps, list(range(N_CORES)))
    return np.concatenate([res.results[i]["y"] for i in range(N_CORES)])
